# revision 15
# baseline (speedup 1.0000x reference)
"""Trainium2 Bass kernel for nn_DC_CRD_85779086836063 (gnn_message_passing).

Reference math (B,C,H,W = 32,64,128,128):
    wvec = mean(x, (2,3))                          # [B, C]
    diff = wvec[:,:,None] - wvec[:,None,:]         # [B, C, C]
    e = exp(-diff); T = |1 - e/(1+e)| - 1          # = sigmoid(diff) - 1
    A = 0.5*(T + T^T) * theta                      # sigmoid(d)+sigmoid(-d) = 1
                                                   # => T + T^T = -1 (exactly)
                                                   # => A = -0.5 * theta  (data-independent)
    H = relu(A @ x_flat)                           # [B, C, HW]
    out = (W_lin @ H)^T + b_lin  reshaped raw [HW,C] -> [C,H,W]

So per batch: out[b] (as [HW, C]) = (W_lin @ relu(-0.5 theta @ x[b]))^T + b_lin.

Sharding: pure data parallel, batch dim 32 -> 4 per core across 8 cores;
weights replicated (tiny, preprocessed on host).

Per-core dataflow (2-batch packing to fill 128 partitions, C=64):
    ablk  = -0.5 * blockdiag(theta^T, theta^T)  [128,128] f32r (host-built)
    wblk2 = blockdiag(W_lin^T, W_lin^T)         [128,128] bf16 (host-built)
    per 2048-col chunk (one 1 MiB load):
      for s in 4 (512-col PSUM banks):
        ps1 = ablk.T @ x2[:, s]        (PE, f32r moving 512 wide: 1 cyc/row)
        h   = relu(ps1) scatter        (ACT, psum->sbuf, output cast to bf16)
              scatter puts n-offset m = 16p + j at h col j*128 + p
      for j in 16 (128-col h blocks):
        ps2[j] = h_j.T @ wblk2         (PE bf16 128-wide: 1 cyc/row; the
                                        LDWEIGHTS of h_j IS the transpose)
        -> psum2[p, b*64+c] = out[b][n0+16p+j][c]
      for s, b: o_b <- ps2 + biasmat   (DVE tensor_tensor add, psum->sbuf)
      2 stores (SWDGE on gpsimd): partition p owns DRAM rows [16p,16p+16)
      of the chunk -> 4 KiB contiguous runs.

Engine budget/core: DMA 2x16.8 MB ~ 93 us (roofline), PE ~27-54 us,
ACT ~36 us, DVE ~52 us, Pool ~38 us. Expect DMA-bound ~95-105 us.
"""

import os
import sys

sys.path.insert(0, "/opt/trn_rl_repo")

import numpy as np
import ml_dtypes

import concourse.bacc as bacc
import concourse.mybir as mybir
from concourse import tile
from concourse.bass_utils import run_bass_kernel_spmd

dt = mybir.dt
AF = mybir.ActivationFunctionType
ALU = mybir.AluOpType

B, C, H, W = 32, 64, 128, 128
HW = H * W
NCORES = 8
BL = B // NCORES  # batches per core
PAIRS = BL // 2

DMACHUNK = 2048  # cols per DMA chunk (1 MiB per load)
SUB = 512  # cols per matmul / PSUM bank
R = DMACHUNK // 128  # rows per partition in the output chunk (16)

VARIANT = "v2"


def _build(variant: str = "v2"):
    nc = bacc.Bacc("TRN2", target_bir_lowering=False, debug=False)

    # f32r is bit-identical to f32 (dt.np(float32r) == np.float32); declaring
    # the DRAM/SBUF tensors as f32r makes the PE run 1 cyc/row on >=256-wide
    # moving operands with no cast pass.
    x_d = nc.dram_tensor("x", [BL, C, HW], dt.float32r, kind="ExternalInput")
    ab_d = nc.dram_tensor("ablk", [128, 128], dt.float32r, kind="ExternalInput")
    wb_d = nc.dram_tensor("wblk2", [128, 128], dt.bfloat16, kind="ExternalInput")
    bm_d = nc.dram_tensor("biasmat", [128, 8 * C], dt.float32, kind="ExternalInput")
    out_d = nc.dram_tensor("out", [BL, HW, C], dt.float32, kind="ExternalOutput")

    with tile.TileContext(nc) as tc:
        with (
            tc.tile_pool(name="const", bufs=1) as const,
            tc.tile_pool(name="xp", bufs=3) as xp,
            tc.tile_pool(name="hp", bufs=3) as hp,
            tc.tile_pool(name="op", bufs=3) as op_,
            tc.tile_pool(name="ps1p", bufs=3, space="PSUM") as ps1p,
            tc.tile_pool(name="ps2p", bufs=4, space="PSUM") as ps2p,
        ):
            # ---------------- constants (host-precomputed) ----------------
            ablk = const.tile([128, 128], dt.float32r, tag="ablk")
            wblk = const.tile([128, 128], dt.bfloat16, tag="wblk")
            biasm = const.tile([128, 8 * C], dt.float32, tag="biasm")
            nc.sync.dma_start(ablk[:], ab_d[:])
            nc.sync.dma_start(wblk[:], wb_d[:])
            nc.sync.dma_start(biasm[:], bm_d[:])

            # ---------------- main loop ----------------
            # bank-local scatter: bank s's 512 n-values go only to mm2
            # blocks 4s..4s+3 (block j = 4s + q%4, partition p = q//4, i.e.
            # n = n0 + 512s + 4p + jj). This makes relu_s -> mm2(4s..) ->
            # TT_s a dep-clean per-bank pipeline; partition p owns 4
            # consecutive DRAM rows per bank (1 KiB contiguous runs).
            xsrc = x_d[:].rearrange("b c n -> (b c) n")
            for pair in range(PAIRS):
                b0 = 2 * pair
                for ci in range(HW // DMACHUNK):
                    n0 = ci * DMACHUNK
                    # load in 2 halves so mm1 of bank 0 starts ~1.5us earlier
                    x2 = xp.tile([128, DMACHUNK], dt.float32r, tag="x2")
                    HALF = DMACHUNK // 2
                    for li in range(2):
                        nc.sync.dma_start(
                            x2[:, li * HALF : (li + 1) * HALF],
                            xsrc[
                                b0 * C : (b0 + 2) * C,
                                n0 + li * HALF : n0 + (li + 1) * HALF,
                            ],
                        )
                    h = hp.tile([128, DMACHUNK], dt.bfloat16, tag="h")
                    # single staging tile, both batches interleaved exactly as
                    # psum2 banks produce them: col = 512s + 128jj + 64bi + c
                    o2 = op_.tile([128, DMACHUNK], dt.float32, tag="o2")
                    NSUB = DMACHUNK // SUB

                    def do_mm1(s):
                        ps1 = ps1p.tile([128, SUB], dt.float32, tag="ps1")
                        nc.tensor.matmul(
                            ps1[:],
                            ablk[:],
                            x2[:, s * SUB : (s + 1) * SUB],
                            start=True,
                            stop=True,
                        )
                        # relu (+ cast to bf16): identity mapping, h[:, m]
                        # holds n-offset m; plain contiguous ACT copy
                        nc.scalar.activation(
                            h[:, s * SUB : (s + 1) * SUB], ps1[:], AF.Relu
                        )

                    def do_mm2(s):
                        ps2 = ps2p.tile([128, SUB], dt.float32, tag="ps2")
                        for jj in range(SUB // 128):
                            j = 4 * s + jj
                            nc.tensor.matmul(
                                ps2[:, jj * 128 : (jj + 1) * 128],
                                h[:, j * 128 : (j + 1) * 128],
                                wblk[:],
                                start=True,
                                stop=True,
                            )
                        # eviction + bias add: one contiguous [128,512] TT
                        # per bank on DVE (gpsimd cannot read PSUM)
                        nc.vector.tensor_tensor(
                            o2[:, s * SUB : (s + 1) * SUB],
                            ps2[:],
                            biasm[:],
                            op=ALU.add,
                        )

                    # software-pipeline PE by 2 banks so mm2_s never stalls
                    # on relu_s (relu latency hides behind mm1_{s+1,s+2})
                    do_mm1(0)
                    do_mm1(1)
                    do_mm1(2)
                    do_mm2(0)
                    do_mm1(3)
                    do_mm2(1)
                    do_mm2(2)
                    do_mm2(3)
                    # stores: psum2 block j partition p holds chunk row
                    # 128j + p -> DRAM 3D AP [p][16 j, stride 128 rows][64 c]
                    # (256 B runs). b0 via ACT HWDGE, b1 via Pool SWDGE ->
                    # Sync queue stays loads-only.
                    o2v = o2[:].rearrange("p (j b c) -> p j b c", j=16, b=2)
                    for bi, eng in ((0, nc.scalar), (1, nc.gpsimd)):
                        dd = out_d[b0 + bi, n0 : n0 + DMACHUNK, :].rearrange(
                            "(j p) c -> p j c", p=128
                        )
                        eng.dma_start(dd, o2v[:, :, bi, :])

    nc.compile()
    return nc


def _ensure_ntff_hook():
    """Register the axon NTFF profile hook (profiling only; best-effort)."""
    import contextlib
    import ctypes
    import types

    if "antenv.axon_hooks" in sys.modules:
        return
    so_path = "/opt/axon/libaxon_pjrt.so"
    try:
        lib = ctypes.CDLL(so_path)
        lib.axon_start_nrt_profile.argtypes = [
            ctypes.POINTER(ctypes.c_int64),
            ctypes.c_size_t,
        ]
        lib.axon_start_nrt_profile.restype = ctypes.c_int64
        lib.axon_stop_nrt_profile.argtypes = [ctypes.c_char_p]
        lib.axon_stop_nrt_profile.restype = ctypes.c_int64
    except (OSError, AttributeError):
        lib = None

    @contextlib.contextmanager
    def _hook(output_dir, device_ids):
        import jax

        jax.devices()
        if device_ids:
            ids = (ctypes.c_int64 * len(device_ids))(*device_ids)
            rc = lib.axon_start_nrt_profile(ids, len(device_ids))
        else:
            rc = lib.axon_start_nrt_profile(None, 0)
        if rc != 0:
            raise RuntimeError(f"axon_start_nrt_profile rc={rc}")
        try:
            yield
        finally:
            n = lib.axon_stop_nrt_profile(str(output_dir).encode())
            print(f"ntff profile: {n} file(s) written to {output_dir}")

    hook = _hook if lib is not None else None
    mod = types.ModuleType("antenv.axon_hooks")
    mod.get_axon_ntff_profile_hook = lambda: hook
    mod.set_axon_ntff_profile_hook = lambda h: None
    sys.modules["antenv.axon_hooks"] = mod


_NC_CACHE = {}


def _get_nc(variant: str):
    if variant not in _NC_CACHE:
        _NC_CACHE[variant] = _build(variant)
    return _NC_CACHE[variant]


def _run(inputs: dict, trace: bool = False, variant: str | None = None):
    variant = variant or VARIANT
    if trace:
        _ensure_ntff_hook()
    nc = _get_nc(variant)
    x = np.ascontiguousarray(inputs["x"], dtype=np.float32)
    theta = np.asarray(inputs["theta"], dtype=np.float32)
    w_lin = np.asarray(inputs["W_lin"], dtype=np.float32)
    b_lin = np.asarray(inputs["b_lin"], dtype=np.float32)

    # host-side preprocessing of the tiny replicated weights
    ablk = np.zeros((128, 128), dtype=np.float32)
    ablk[:C, :C] = -0.5 * theta.T
    ablk[C:, C:] = -0.5 * theta.T
    wblk2 = np.zeros((128, 128), dtype=np.float32)
    wblk2[:C, :C] = w_lin.T
    wblk2[C:, C:] = w_lin.T
    wblk2 = wblk2.astype(ml_dtypes.bfloat16)
    biasmat = np.tile(b_lin, (128, 8)).astype(np.float32)

    in_maps = [
        {
            "x": np.ascontiguousarray(x[i * BL : (i + 1) * BL].reshape(BL, C, HW)),
            "ablk": ablk,
            "wblk2": wblk2,
            "biasmat": biasmat,
        }
        for i in range(NCORES)
    ]
    # Occasionally the first execution of a freshly-loaded NEFF fails with
    # NRT_EXEC_UNIT_UNRECOVERABLE; a retry on the recovered device succeeds.
    import time

    last_err = None
    for attempt in range(4):
        try:
            res = run_bass_kernel_spmd(
                nc,
                in_maps,
                core_ids=list(range(NCORES)),
                trace=trace and attempt == 0,
            )
            break
        except Exception as e:  # noqa: BLE001
            last_err = e
            try:  # drop the (possibly dead) PJRT client; next call re-inits
                import jax

                jax.clear_caches()
                jax.extend.backend.clear_backends()
            except Exception:  # noqa: BLE001
                pass
            time.sleep(10 * (attempt + 1))
    else:
        raise last_err
    shards = [r["out"].reshape(BL, C, H, W) for r in res.results]
    return np.concatenate(shards, axis=0), res


def kernel(x, theta, W_lin, b_lin):
    out, _ = _run({"x": x, "theta": theta, "W_lin": W_lin, "b_lin": b_lin})
    return out
